# revision 15
# baseline (speedup 1.0000x reference)
"""Spatial self-attention scores kernel for Trainium2 (8 NeuronCores).

Computes, per batch b:
    qk = W @ x_b          # [256, 4096] = [256,256] @ [256,4096]
    q, k = qk[:128], qk[128:]
    sim = (q.T @ k) * 128**-0.5
    out_b = softmax(sim, axis=-1)        # [4096, 4096]
Output: [8, 1, 4096, 4096] float32.

Sharding: data-parallel over batch, one batch image per NeuronCore.

Per-core pipeline (all phases overlap under the Tile scheduler):
  - x DMA'd in as float32r (SWDGE cast); W transposed on PE via identity.
  - projection matmuls -> q,k in SBUF as [d=128, s=4096] float32r.
  - per 128-query row-tile: 8 matmuls (K=128, N=512) into 4-bank PSUM
    tiles; one ScalarE ACTIVATE per 2048 columns computes exp(SCALE*sim)
    with a fused row-sum (accum_out); DVE combines the partial sums,
    takes the reciprocal, and scales the row in one tensor_scalar.
  - output rows DMA'd out in 4 MB blocks (two row-tiles per transfer).
"""

import numpy as np
from contextlib import ExitStack

import concourse.bass as bass
import concourse.tile as tile
from concourse import bacc, mybir
from concourse.bass_utils import run_bass_kernel_spmd
from concourse.masks import make_identity

B = 8
C = 256
HW = 4096
D = 128
SCALE = D ** -0.5
N_CORES = 8

BANK = 512             # PSUM bank width (fp32) = one matmul free-dim
ACT_CHUNK = 2048       # one ScalarE activation spans 4 banks
N_ACT = HW // ACT_CHUNK          # 2
GRP = 2                # row-tiles per output DMA (2 -> 4 MB transfers)
N_GRP = HW // (128 * GRP)        # 16
OUT_BUFS = 4
X_CHUNK = 1024         # x input DMA granularity (overlaps with projection)

F32 = mybir.dt.float32
# float32r streams through the PE at 2 cycles/row (vs 4 for float32)
# with near-fp32 precision (measured ~3e-4 scale-relative on this
# kernel). The BIR verifier requires fp32r operands to be *produced*
# as fp32r, so operand tiles carry this dtype and their producers
# (SWDGE cast DMA / DVE copies) write it directly.
MM_DT = mybir.dt.float32r


def _emit(ctx: ExitStack, tc: tile.TileContext, out_ap, x_ap, w_ap):
    nc = tc.nc

    const = ctx.enter_context(tc.tile_pool(name="const", bufs=1))
    data = ctx.enter_context(tc.tile_pool(name="data", bufs=1))
    psum = ctx.enter_context(tc.tile_pool(name="psum", bufs=2, space="PSUM"))
    small = ctx.enter_context(tc.tile_pool(name="small", bufs=4))

    ident = const.tile([128, 128], F32)
    make_identity(nc, ident)

    # ---- W [256, 256] -> SBUF as [p, o_tile, c]
    w_sb = const.tile([128, 2, C], F32)
    nc.sync.dma_start(out=w_sb, in_=w_ap.rearrange("(t p) c -> p t c", p=128))

    # ---- PE warm-up: ~7 us of throwaway matmuls while x is loading.
    # The PE clock gate (HAM) only releases to 2.4 GHz after ~3.4 us of
    # sustained activity; warming during the input DMA makes the
    # projection and the first attention row-tiles run at full rate.
    warm_f32 = const.tile([128, BANK], F32)
    nc.vector.memset(warm_f32, 0.0)
    warm = const.tile([128, BANK], MM_DT)
    nc.vector.tensor_copy(out=warm, in_=warm_f32)
    wps = psum.tile([128, ACT_CHUNK], F32, tag="ps")
    for _ in range(8):
        nc.tensor.matmul(
            wps[:, 0:BANK], warm[:, 0:128], warm, start=True, stop=True
        )

    # ---- transpose W on PE -> wt_sb[c_sub, c_tile, o] (contraction c on partitions)
    wt_sb = const.tile([128, 2, 2 * D], MM_DT)
    for t in range(2):          # output-channel tile (q half / k half)
        for ct in range(2):     # input-channel tile
            ps = psum.tile([128, ACT_CHUNK], F32, tag="ps")
            nc.tensor.transpose(
                ps[:, 0:128], w_sb[:, t, ct * 128:(ct + 1) * 128], ident
            )
            nc.vector.tensor_copy(
                out=wt_sb[:, ct, t * 128:(t + 1) * 128], in_=ps[:, 0:128]
            )

    q_sb = data.tile([128, HW], MM_DT)
    k_sb = data.tile([128, HW], MM_DT)

    with tc.tile_pool(name="xpool", bufs=1) as xp:
        # x loaded with an SWDGE cast straight to fp32r, split so the
        # projection can start on the first half while the second lands
        x_sb = xp.tile([128, 2, HW], MM_DT)
        x_view = x_ap.rearrange("(t p) s -> p t s", p=128)
        for a in range(HW // X_CHUNK):
            sl = slice(a * X_CHUNK, (a + 1) * X_CHUNK)
            nc.gpsimd.dma_start(out=x_sb[:, :, sl], in_=x_view[:, :, sl])

        # ---- projection: k first (every attention row needs all of k),
        # then q chunk by chunk so row-tile 0 unblocks after q's first chunk
        for t, dst in ((1, k_sb), (0, q_sb)):
            for a in range(N_ACT):
                ps = psum.tile([128, ACT_CHUNK], F32, tag="ps")
                for jj in range(ACT_CHUNK // BANK):
                    sl = slice(a * ACT_CHUNK + jj * BANK,
                               a * ACT_CHUNK + (jj + 1) * BANK)
                    bk = slice(jj * BANK, (jj + 1) * BANK)
                    nc.tensor.matmul(
                        ps[:, bk], wt_sb[:, 0, t * 128:(t + 1) * 128],
                        x_sb[:, 0, sl], start=True, stop=False,
                    )
                    nc.tensor.matmul(
                        ps[:, bk], wt_sb[:, 1, t * 128:(t + 1) * 128],
                        x_sb[:, 1, sl], start=False, stop=True,
                    )
                nc.vector.tensor_copy(
                    out=dst[:, a * ACT_CHUNK:(a + 1) * ACT_CHUNK], in_=ps
                )

    # output staging opens after xpool closes so SBUF is reused
    outp = ctx.enter_context(tc.tile_pool(name="outp", bufs=OUT_BUFS))

    # ---- attention rows: groups of GRP row-tiles (128 queries each)
    out_view = out_ap.rearrange("(g t p) m -> g p t m", t=GRP, p=128)
    for g in range(N_GRP):
        out_grp = outp.tile([128, GRP, HW], F32, tag="out")
        for t in range(GRP):
            i = g * GRP + t
            lhs = q_sb[:, i * 128:(i + 1) * 128]
            sums = small.tile([128, N_ACT], F32, tag="sums")
            for a in range(N_ACT):
                ps = psum.tile([128, ACT_CHUNK], F32, tag="ps")
                for jj in range(ACT_CHUNK // BANK):
                    sl = slice(a * ACT_CHUNK + jj * BANK,
                               a * ACT_CHUNK + (jj + 1) * BANK)
                    nc.tensor.matmul(
                        ps[:, jj * BANK:(jj + 1) * BANK], lhs, k_sb[:, sl],
                        start=True, stop=True,
                    )
                # exp(SCALE * sim) with fused partial row-sum
                nc.scalar.activation(
                    out=out_grp[:, t, a * ACT_CHUNK:(a + 1) * ACT_CHUNK],
                    in_=ps,
                    func=mybir.ActivationFunctionType.Exp,
                    scale=SCALE,
                    accum_out=sums[:, a:a + 1],
                )
            rsum = small.tile([128, 1], F32, tag="rsum")
            nc.vector.tensor_reduce(
                out=rsum, in_=sums, axis=mybir.AxisListType.X,
                op=mybir.AluOpType.add,
            )
            recip = small.tile([128, 1], F32, tag="recip")
            nc.vector.reciprocal(out=recip, in_=rsum)
            nc.vector.tensor_scalar_mul(
                out=out_grp[:, t, :], in0=out_grp[:, t, :], scalar1=recip
            )
        nc.sync.dma_start(out=out_view[g], in_=out_grp)


_built = None


def _get_nc():
    global _built
    if _built is None:
        nc = bacc.Bacc("TRN2", target_bir_lowering=False, debug=False)
        x = nc.dram_tensor("x", [C, HW], F32, kind="ExternalInput").ap()
        w = nc.dram_tensor("w", [2 * D, C], F32, kind="ExternalInput").ap()
        out = nc.dram_tensor("out", [HW, HW], F32, kind="ExternalOutput").ap()
        with tile.TileContext(nc) as tc:
            with ExitStack() as ctx:
                _emit(ctx, tc, out, x, w)
        nc.compile()
        _built = nc
    return _built


def kernel(x: np.ndarray, W: np.ndarray) -> np.ndarray:
    nc = _get_nc()
    x = np.asarray(x, dtype=np.float32)
    W = np.ascontiguousarray(np.asarray(W, dtype=np.float32))
    in_maps = [
        {"x": np.ascontiguousarray(x[b].reshape(C, HW)), "w": W} for b in range(B)
    ]
    res = run_bass_kernel_spmd(nc, in_maps, core_ids=list(range(N_CORES)))
    out = np.stack([res.results[b]["out"] for b in range(B)])
    return out[:, None]


# revision 16
# speedup vs baseline: 1.0122x; 1.0122x over previous
"""Spatial self-attention scores kernel for Trainium2 (8 NeuronCores).

Computes, per batch b:
    qk = W @ x_b          # [256, 4096] = [256,256] @ [256,4096]
    q, k = qk[:128], qk[128:]
    sim = (q.T @ k) * 128**-0.5
    out_b = softmax(sim, axis=-1)        # [4096, 4096]
Output: [8, 1, 4096, 4096] float32.

Sharding: data-parallel over batch, one batch image per NeuronCore.

Per-core pipeline (all phases overlap under the Tile scheduler):
  - x DMA'd in as float32r (SWDGE cast); W transposed on PE via identity.
  - projection matmuls -> q,k in SBUF as [d=128, s=4096] float32r.
  - per 128-query row-tile: 8 matmuls (K=128, N=512) into 4-bank PSUM
    tiles; one ScalarE ACTIVATE per 2048 columns computes exp(SCALE*sim)
    with a fused row-sum (accum_out); DVE combines the partial sums,
    takes the reciprocal, and scales the row in one tensor_scalar.
  - output rows DMA'd out in 4 MB blocks (two row-tiles per transfer).
"""

import numpy as np
from contextlib import ExitStack

import concourse.bass as bass
import concourse.tile as tile
from concourse import bacc, mybir
from concourse.bass_utils import run_bass_kernel_spmd
from concourse.masks import make_identity

B = 8
C = 256
HW = 4096
D = 128
SCALE = D ** -0.5
N_CORES = 8

BANK = 512             # PSUM bank width (fp32) = one matmul free-dim
ACT_CHUNK = 2048       # one ScalarE activation spans 4 banks
N_ACT = HW // ACT_CHUNK          # 2
GRP = 2                # row-tiles per output DMA (2 -> 4 MB transfers)
N_GRP = HW // (128 * GRP)        # 16
OUT_BUFS = 4
X_CHUNK = 1024         # x input DMA granularity (overlaps with projection)

F32 = mybir.dt.float32
# float32r streams through the PE at 2 cycles/row (vs 4 for float32)
# with near-fp32 precision (measured ~3e-4 scale-relative on this
# kernel). The BIR verifier requires fp32r operands to be *produced*
# as fp32r, so operand tiles carry this dtype and their producers
# (SWDGE cast DMA / DVE copies) write it directly.
MM_DT = mybir.dt.float32r


def _emit(ctx: ExitStack, tc: tile.TileContext, out_ap, x_ap, w_ap):
    nc = tc.nc

    const = ctx.enter_context(tc.tile_pool(name="const", bufs=1))
    data = ctx.enter_context(tc.tile_pool(name="data", bufs=1))
    psum = ctx.enter_context(tc.tile_pool(name="psum", bufs=2, space="PSUM"))
    small = ctx.enter_context(tc.tile_pool(name="small", bufs=4))

    ident = const.tile([128, 128], F32)
    make_identity(nc, ident)

    # ---- W [256, 256] -> SBUF as [p, o_tile, c]
    w_sb = const.tile([128, 2, C], F32)
    nc.sync.dma_start(out=w_sb, in_=w_ap.rearrange("(t p) c -> p t c", p=128))

    # ---- PE warm-up: ~7 us of throwaway matmuls while x is loading.
    # The PE clock gate (HAM) only releases to 2.4 GHz after ~3.4 us of
    # sustained activity; warming during the input DMA makes the
    # projection and the first attention row-tiles run at full rate.
    warm_f32 = const.tile([128, BANK], F32)
    nc.vector.memset(warm_f32, 0.0)
    warm = const.tile([128, BANK], MM_DT)
    nc.vector.tensor_copy(out=warm, in_=warm_f32)
    wps = psum.tile([128, ACT_CHUNK], F32, tag="ps")
    for _ in range(8):
        nc.tensor.matmul(
            wps[:, 0:BANK], warm[:, 0:128], warm, start=True, stop=True
        )

    # ---- transpose W on PE -> wt_sb[c_sub, c_tile, o] (contraction c on partitions)
    wt_sb = const.tile([128, 2, 2 * D], MM_DT)
    for t in range(2):          # output-channel tile (q half / k half)
        for ct in range(2):     # input-channel tile
            ps = psum.tile([128, ACT_CHUNK], F32, tag="ps")
            nc.tensor.transpose(
                ps[:, 0:128], w_sb[:, t, ct * 128:(ct + 1) * 128], ident
            )
            nc.vector.tensor_copy(
                out=wt_sb[:, ct, t * 128:(t + 1) * 128], in_=ps[:, 0:128]
            )

    q_sb = data.tile([128, HW], MM_DT)
    k_sb = data.tile([128, HW], MM_DT)

    with tc.tile_pool(name="xpool", bufs=1) as xp:
        # x loaded with an SWDGE cast straight to fp32r, split so the
        # projection can start on the first half while the second lands
        x_sb = xp.tile([128, 2, HW], MM_DT)
        x_view = x_ap.rearrange("(t p) s -> p t s", p=128)
        for a in range(HW // X_CHUNK):
            sl = slice(a * X_CHUNK, (a + 1) * X_CHUNK)
            nc.gpsimd.dma_start(out=x_sb[:, :, sl], in_=x_view[:, :, sl])

        # ---- projection. Order k0, q0, k1, q1: the first attention
        # row-tiles need k chunk 0 + q chunk 0 first. Copies out of PSUM
        # are per-bank so they pipeline with the matmuls.
        for t, dst, a in ((1, k_sb, 0), (0, q_sb, 0), (1, k_sb, 1), (0, q_sb, 1)):
            ps = psum.tile([128, ACT_CHUNK], F32, tag="ps")
            for jj in range(ACT_CHUNK // BANK):
                sl = slice(a * ACT_CHUNK + jj * BANK,
                           a * ACT_CHUNK + (jj + 1) * BANK)
                bk = slice(jj * BANK, (jj + 1) * BANK)
                nc.tensor.matmul(
                    ps[:, bk], wt_sb[:, 0, t * 128:(t + 1) * 128],
                    x_sb[:, 0, sl], start=True, stop=False,
                )
                nc.tensor.matmul(
                    ps[:, bk], wt_sb[:, 1, t * 128:(t + 1) * 128],
                    x_sb[:, 1, sl], start=False, stop=True,
                )
                nc.vector.tensor_copy(out=dst[:, sl], in_=ps[:, bk])

    # output staging opens after xpool closes so SBUF is reused
    outp = ctx.enter_context(tc.tile_pool(name="outp", bufs=OUT_BUFS))

    # ---- attention rows: groups of GRP row-tiles (128 queries each)
    out_view = out_ap.rearrange("(g t p) m -> g p t m", t=GRP, p=128)
    for g in range(N_GRP):
        out_grp = outp.tile([128, GRP, HW], F32, tag="out")
        for t in range(GRP):
            i = g * GRP + t
            lhs = q_sb[:, i * 128:(i + 1) * 128]
            sums = small.tile([128, N_ACT], F32, tag="sums")
            for a in range(N_ACT):
                ps = psum.tile([128, ACT_CHUNK], F32, tag="ps")
                for jj in range(ACT_CHUNK // BANK):
                    sl = slice(a * ACT_CHUNK + jj * BANK,
                               a * ACT_CHUNK + (jj + 1) * BANK)
                    nc.tensor.matmul(
                        ps[:, jj * BANK:(jj + 1) * BANK], lhs, k_sb[:, sl],
                        start=True, stop=True,
                    )
                # exp(SCALE * sim) with fused partial row-sum
                nc.scalar.activation(
                    out=out_grp[:, t, a * ACT_CHUNK:(a + 1) * ACT_CHUNK],
                    in_=ps,
                    func=mybir.ActivationFunctionType.Exp,
                    scale=SCALE,
                    accum_out=sums[:, a:a + 1],
                )
            rsum = small.tile([128, 1], F32, tag="rsum")
            nc.vector.tensor_reduce(
                out=rsum, in_=sums, axis=mybir.AxisListType.X,
                op=mybir.AluOpType.add,
            )
            recip = small.tile([128, 1], F32, tag="recip")
            nc.vector.reciprocal(out=recip, in_=rsum)
            nc.vector.tensor_scalar_mul(
                out=out_grp[:, t, :], in0=out_grp[:, t, :], scalar1=recip
            )
        nc.sync.dma_start(out=out_view[g], in_=out_grp)


_built = None


def _get_nc():
    global _built
    if _built is None:
        nc = bacc.Bacc("TRN2", target_bir_lowering=False, debug=False)
        x = nc.dram_tensor("x", [C, HW], F32, kind="ExternalInput").ap()
        w = nc.dram_tensor("w", [2 * D, C], F32, kind="ExternalInput").ap()
        out = nc.dram_tensor("out", [HW, HW], F32, kind="ExternalOutput").ap()
        with tile.TileContext(nc) as tc:
            with ExitStack() as ctx:
                _emit(ctx, tc, out, x, w)
        nc.compile()
        _built = nc
    return _built


def kernel(x: np.ndarray, W: np.ndarray) -> np.ndarray:
    nc = _get_nc()
    x = np.asarray(x, dtype=np.float32)
    W = np.ascontiguousarray(np.asarray(W, dtype=np.float32))
    in_maps = [
        {"x": np.ascontiguousarray(x[b].reshape(C, HW)), "w": W} for b in range(B)
    ]
    res = run_bass_kernel_spmd(nc, in_maps, core_ids=list(range(N_CORES)))
    out = np.stack([res.results[b]["out"] for b in range(B)])
    return out[:, None]


# revision 20
# speedup vs baseline: 1.0146x; 1.0024x over previous
"""Spatial self-attention scores kernel for Trainium2 (8 NeuronCores).

Computes, per batch b:
    qk = W @ x_b          # [256, 4096] = [256,256] @ [256,4096]
    q, k = qk[:128], qk[128:]
    sim = (q.T @ k) * 128**-0.5
    out_b = softmax(sim, axis=-1)        # [4096, 4096]
Output: [8, 1, 4096, 4096] float32.

Sharding: data-parallel over batch, one batch image per NeuronCore.

Per-core pipeline (all phases overlap under the Tile scheduler):
  - x DMA'd in as float32r (SWDGE cast); W transposed on PE via identity.
  - projection matmuls -> q,k in SBUF as [d=128, s=4096] float32r.
  - per 128-query row-tile: 8 matmuls (K=128, N=512) into 4-bank PSUM
    tiles; one ScalarE ACTIVATE per 2048 columns computes exp(SCALE*sim)
    with a fused row-sum (accum_out); DVE combines the partial sums,
    takes the reciprocal, and scales the row in one tensor_scalar.
  - output rows DMA'd out in 4 MB blocks (two row-tiles per transfer).
"""

import numpy as np
from contextlib import ExitStack

import concourse.bass as bass
import concourse.tile as tile
from concourse import bacc, mybir
from concourse.bass_utils import run_bass_kernel_spmd
from concourse.masks import make_identity

B = 8
C = 256
HW = 4096
D = 128
SCALE = D ** -0.5
N_CORES = 8

BANK = 512             # PSUM bank width (fp32) = one matmul free-dim
ACT_CHUNK = 2048       # one ScalarE activation spans 4 banks
N_ACT = HW // ACT_CHUNK          # 2
GRP = 2                # row-tiles per output DMA (2 -> 4 MB transfers)
N_GRP = HW // (128 * GRP)        # 16
OUT_BUFS = 4
X_CHUNK = 1024         # x input DMA granularity (overlaps with projection)

F32 = mybir.dt.float32
# float32r streams through the PE at 2 cycles/row (vs 4 for float32)
# with near-fp32 precision (measured ~3e-4 scale-relative on this
# kernel). The BIR verifier requires fp32r operands to be *produced*
# as fp32r, so operand tiles carry this dtype and their producers
# (SWDGE cast DMA / DVE copies) write it directly.
MM_DT = mybir.dt.float32r


def _emit(ctx: ExitStack, tc: tile.TileContext, out_ap, x_ap, w_ap):
    nc = tc.nc

    const = ctx.enter_context(tc.tile_pool(name="const", bufs=1))
    data = ctx.enter_context(tc.tile_pool(name="data", bufs=1))
    psum = ctx.enter_context(tc.tile_pool(name="psum", bufs=2, space="PSUM"))
    small = ctx.enter_context(tc.tile_pool(name="small", bufs=4))

    ident = const.tile([128, 128], F32)
    make_identity(nc, ident)

    # ---- W [256, 256] -> SBUF as [p, o_tile, c]
    w_sb = const.tile([128, 2, C], F32)
    nc.sync.dma_start(out=w_sb, in_=w_ap.rearrange("(t p) c -> p t c", p=128))

    # ---- PE warm-up: ~7 us of throwaway matmuls while x is loading.
    # The PE clock gate (HAM) only releases to 2.4 GHz after ~3.4 us of
    # sustained activity; warming during the input DMA makes the
    # projection and the first attention row-tiles run at full rate.
    warm_f32 = const.tile([128, BANK], F32)
    nc.vector.memset(warm_f32, 0.0)
    warm = const.tile([128, BANK], MM_DT)
    nc.vector.tensor_copy(out=warm, in_=warm_f32)
    wps = psum.tile([128, ACT_CHUNK], F32, tag="ps")
    for _ in range(5):
        nc.tensor.matmul(
            wps[:, 0:BANK], warm[:, 0:128], warm, start=True, stop=True
        )
    # pull the exp table load off the first real activation
    tbl = small.tile([128, 1], F32, tag="tbl")
    nc.scalar.activation(
        out=tbl, in_=warm_f32[:, 0:1], func=mybir.ActivationFunctionType.Exp
    )

    # ---- transpose W on PE -> wt_sb[c_sub, c_tile, o] (contraction c on partitions)
    wt_sb = const.tile([128, 2, 2 * D], MM_DT)
    for t in range(2):          # output-channel tile (q half / k half)
        for ct in range(2):     # input-channel tile
            ps = psum.tile([128, ACT_CHUNK], F32, tag="ps")
            nc.tensor.transpose(
                ps[:, 0:128], w_sb[:, t, ct * 128:(ct + 1) * 128], ident
            )
            nc.vector.tensor_copy(
                out=wt_sb[:, ct, t * 128:(t + 1) * 128], in_=ps[:, 0:128]
            )

    q_sb = data.tile([128, HW], MM_DT)
    k_sb = data.tile([128, HW], MM_DT)

    def proj_chunk(t, dst, a, x_half):
        """Project output-channel half t for column chunk a; x_half holds
        x columns [a*ACT_CHUNK, (a+1)*ACT_CHUNK)."""
        ps = psum.tile([128, ACT_CHUNK], F32, tag="ps")
        for jj in range(ACT_CHUNK // BANK):
            sl = slice(a * ACT_CHUNK + jj * BANK,
                       a * ACT_CHUNK + (jj + 1) * BANK)
            lo = slice(jj * BANK, (jj + 1) * BANK)
            nc.tensor.matmul(
                ps[:, lo], wt_sb[:, 0, t * 128:(t + 1) * 128],
                x_half[:, 0, lo], start=True, stop=False,
            )
            nc.tensor.matmul(
                ps[:, lo], wt_sb[:, 1, t * 128:(t + 1) * 128],
                x_half[:, 1, lo], start=False, stop=True,
            )
            nc.vector.tensor_copy(out=dst[:, sl], in_=ps[:, lo])

    outp = None
    out_view = out_ap.rearrange("(g t p) m -> g p t m", t=GRP, p=128)

    def emit_group(g):
        out_grp = outp.tile([128, GRP, HW], F32, tag="out")
        for t in range(GRP):
            i = g * GRP + t
            lhs = q_sb[:, i * 128:(i + 1) * 128]
            sums = small.tile([128, N_ACT], F32, tag="sums")
            for a in range(N_ACT):
                ps = psum.tile([128, ACT_CHUNK], F32, tag="ps")
                for jj in range(ACT_CHUNK // BANK):
                    sl = slice(a * ACT_CHUNK + jj * BANK,
                               a * ACT_CHUNK + (jj + 1) * BANK)
                    nc.tensor.matmul(
                        ps[:, jj * BANK:(jj + 1) * BANK], lhs, k_sb[:, sl],
                        start=True, stop=True,
                    )
                # exp(SCALE * sim) with fused partial row-sum
                nc.scalar.activation(
                    out=out_grp[:, t, a * ACT_CHUNK:(a + 1) * ACT_CHUNK],
                    in_=ps,
                    func=mybir.ActivationFunctionType.Exp,
                    scale=SCALE,
                    accum_out=sums[:, a:a + 1],
                )
            rsum = small.tile([128, 1], F32, tag="rsum")
            nc.vector.tensor_reduce(
                out=rsum, in_=sums, axis=mybir.AxisListType.X,
                op=mybir.AluOpType.add,
            )
            recip = small.tile([128, 1], F32, tag="recip")
            nc.vector.reciprocal(out=recip, in_=rsum)
            nc.vector.tensor_scalar_mul(
                out=out_grp[:, t, :], in0=out_grp[:, t, :], scalar1=recip
            )
        nc.sync.dma_start(out=out_view[g], in_=out_grp)

    # x loaded with an SWDGE cast straight to fp32r, in halves: the
    # second half lives in the long-lived pool (16 KB/partition), the
    # first half in a scoped pool freed before output staging opens.
    x_view = x_ap.rearrange("(t p) s -> p t s", p=128)
    x1_sb = data.tile([128, 2, ACT_CHUNK], MM_DT)
    with tc.tile_pool(name="xpool", bufs=1) as xp:
        x0_sb = xp.tile([128, 2, ACT_CHUNK], MM_DT)
        for half, dst_x in ((0, x0_sb), (1, x1_sb)):
            for c in range(ACT_CHUNK // X_CHUNK):
                src = slice(half * ACT_CHUNK + c * X_CHUNK,
                            half * ACT_CHUNK + (c + 1) * X_CHUNK)
                loc = slice(c * X_CHUNK, (c + 1) * X_CHUNK)
                nc.gpsimd.dma_start(
                    out=dst_x[:, :, loc], in_=x_view[:, :, src]
                )

        # ---- projection, interleaved with the first attention groups.
        # Groups 0-7 read only q chunk 0, so q chunk 1 is deferred until
        # after them: the PE (in-order) reaches the first sim matmuls and
        # the first output DMA sooner.
        proj_chunk(1, k_sb, 0, x0_sb)   # k cols 0:2048
        proj_chunk(0, q_sb, 0, x0_sb)   # q rows 0:2048 (row-tiles 0-15)
        proj_chunk(1, k_sb, 1, x1_sb)   # k cols 2048:4096

    outp = ctx.enter_context(tc.tile_pool(name="outp", bufs=OUT_BUFS))
    for g in range(N_GRP // 2):
        emit_group(g)
    proj_chunk(0, q_sb, 1, x1_sb)       # q rows 2048:4096 (row-tiles 16-31)
    for g in range(N_GRP // 2, N_GRP):
        emit_group(g)


_built = None


def _get_nc():
    global _built
    if _built is None:
        nc = bacc.Bacc("TRN2", target_bir_lowering=False, debug=False)
        x = nc.dram_tensor("x", [C, HW], F32, kind="ExternalInput").ap()
        w = nc.dram_tensor("w", [2 * D, C], F32, kind="ExternalInput").ap()
        out = nc.dram_tensor("out", [HW, HW], F32, kind="ExternalOutput").ap()
        with tile.TileContext(nc) as tc:
            with ExitStack() as ctx:
                _emit(ctx, tc, out, x, w)
        nc.compile()
        _built = nc
    return _built


def kernel(x: np.ndarray, W: np.ndarray) -> np.ndarray:
    nc = _get_nc()
    x = np.asarray(x, dtype=np.float32)
    W = np.ascontiguousarray(np.asarray(W, dtype=np.float32))
    in_maps = [
        {"x": np.ascontiguousarray(x[b].reshape(C, HW)), "w": W} for b in range(B)
    ]
    res = run_bass_kernel_spmd(nc, in_maps, core_ids=list(range(N_CORES)))
    out = np.stack([res.results[b]["out"] for b in range(B)])
    return out[:, None]


# revision 23
# speedup vs baseline: 1.0265x; 1.0118x over previous
"""Spatial self-attention scores kernel for Trainium2 (8 NeuronCores).

Computes, per batch b:
    qk = W @ x_b          # [256, 4096] = [256,256] @ [256,4096]
    q, k = qk[:128], qk[128:]
    sim = (q.T @ k) * 128**-0.5
    out_b = softmax(sim, axis=-1)        # [4096, 4096]
Output: [8, 1, 4096, 4096] float32.

Sharding: data-parallel over batch, one batch image per NeuronCore.

Per-core pipeline (all phases overlap under the Tile scheduler):
  - x DMA'd in as float32r (SWDGE cast); W transposed on PE via identity.
  - projection matmuls -> q,k in SBUF as [d=128, s=4096] float32r.
  - per 128-query row-tile: 8 matmuls (K=128, N=512) into 4-bank PSUM
    tiles; one ScalarE ACTIVATE per 2048 columns computes exp(SCALE*sim)
    with a fused row-sum (accum_out); DVE combines the partial sums,
    takes the reciprocal, and scales the row in one tensor_scalar.
  - output rows DMA'd out in 4 MB blocks (two row-tiles per transfer).
"""

import numpy as np
from contextlib import ExitStack

import concourse.bass as bass
import concourse.tile as tile
from concourse import bacc, mybir
from concourse.bass_utils import run_bass_kernel_spmd
from concourse.masks import make_identity

B = 8
C = 256
HW = 4096
D = 128
SCALE = D ** -0.5
N_CORES = 8

BANK = 512             # PSUM bank width (fp32) = one matmul free-dim
ACT_CHUNK = 2048       # one ScalarE activation spans 4 banks
N_ACT = HW // ACT_CHUNK          # 2
GRP = 2                # row-tiles per output DMA (2 -> 4 MB transfers)
N_GRP = HW // (128 * GRP)        # 16
OUT_BUFS = 4
X_CHUNK = 1024         # x input DMA granularity (overlaps with projection)
# Dummy activations inserted between output groups. Two NeuronCores
# share one HBM stack; unpaced, each core demands ~480 GB/s, the pair
# oversubscribes the stack and arbitration starves one of them (~40 us
# spread). Pacing the producer loop to ~the fair share equalizes cores.
PACE_ACTS = 4

F32 = mybir.dt.float32
# float32r streams through the PE at 2 cycles/row (vs 4 for float32)
# with near-fp32 precision (measured ~3e-4 scale-relative on this
# kernel). The BIR verifier requires fp32r operands to be *produced*
# as fp32r, so operand tiles carry this dtype and their producers
# (SWDGE cast DMA / DVE copies) write it directly.
MM_DT = mybir.dt.float32r


def _emit(ctx: ExitStack, tc: tile.TileContext, out_ap, x_ap, w_ap):
    nc = tc.nc

    const = ctx.enter_context(tc.tile_pool(name="const", bufs=1))
    data = ctx.enter_context(tc.tile_pool(name="data", bufs=1))
    psum = ctx.enter_context(tc.tile_pool(name="psum", bufs=2, space="PSUM"))
    small = ctx.enter_context(tc.tile_pool(name="small", bufs=4))

    ident = const.tile([128, 128], F32)
    make_identity(nc, ident)

    # ---- W [256, 256] -> SBUF as [p, o_tile, c]
    w_sb = const.tile([128, 2, C], F32)
    nc.sync.dma_start(out=w_sb, in_=w_ap.rearrange("(t p) c -> p t c", p=128))

    # ---- PE warm-up: ~7 us of throwaway matmuls while x is loading.
    # The PE clock gate (HAM) only releases to 2.4 GHz after ~3.4 us of
    # sustained activity; warming during the input DMA makes the
    # projection and the first attention row-tiles run at full rate.
    warm_f32 = const.tile([128, BANK], F32)
    nc.vector.memset(warm_f32, 0.0)
    warm = const.tile([128, BANK], MM_DT)
    nc.vector.tensor_copy(out=warm, in_=warm_f32)
    wps = psum.tile([128, ACT_CHUNK], F32, tag="ps")
    for _ in range(5):
        nc.tensor.matmul(
            wps[:, 0:BANK], warm[:, 0:128], warm, start=True, stop=True
        )
    # pull the exp table load off the first real activation
    tbl = small.tile([128, 1], F32, tag="tbl")
    nc.scalar.activation(
        out=tbl, in_=warm_f32[:, 0:1], func=mybir.ActivationFunctionType.Exp
    )
    pace_out = const.tile([128, BANK], F32)

    def pace(n):
        for _ in range(n):
            nc.scalar.activation(
                out=pace_out, in_=warm_f32,
                func=mybir.ActivationFunctionType.Copy,
            )

    # ---- transpose W on PE -> wt_sb[c_sub, c_tile, o] (contraction c on partitions)
    wt_sb = const.tile([128, 2, 2 * D], MM_DT)
    for t in range(2):          # output-channel tile (q half / k half)
        for ct in range(2):     # input-channel tile
            ps = psum.tile([128, ACT_CHUNK], F32, tag="ps")
            nc.tensor.transpose(
                ps[:, 0:128], w_sb[:, t, ct * 128:(ct + 1) * 128], ident
            )
            nc.vector.tensor_copy(
                out=wt_sb[:, ct, t * 128:(t + 1) * 128], in_=ps[:, 0:128]
            )

    q_sb = data.tile([128, HW], MM_DT)
    k_sb = data.tile([128, HW], MM_DT)

    def proj_chunk(t, dst, a, x_half):
        """Project output-channel half t for column chunk a; x_half holds
        x columns [a*ACT_CHUNK, (a+1)*ACT_CHUNK)."""
        ps = psum.tile([128, ACT_CHUNK], F32, tag="ps")
        for jj in range(ACT_CHUNK // BANK):
            sl = slice(a * ACT_CHUNK + jj * BANK,
                       a * ACT_CHUNK + (jj + 1) * BANK)
            lo = slice(jj * BANK, (jj + 1) * BANK)
            nc.tensor.matmul(
                ps[:, lo], wt_sb[:, 0, t * 128:(t + 1) * 128],
                x_half[:, 0, lo], start=True, stop=False,
            )
            nc.tensor.matmul(
                ps[:, lo], wt_sb[:, 1, t * 128:(t + 1) * 128],
                x_half[:, 1, lo], start=False, stop=True,
            )
            nc.vector.tensor_copy(out=dst[:, sl], in_=ps[:, lo])

    outp = None
    out_view = out_ap.rearrange("(g t p) m -> g p t m", t=GRP, p=128)

    def emit_group(g):
        out_grp = outp.tile([128, GRP, HW], F32, tag="out")
        for t in range(GRP):
            i = g * GRP + t
            lhs = q_sb[:, i * 128:(i + 1) * 128]
            sums = small.tile([128, N_ACT], F32, tag="sums")
            for a in range(N_ACT):
                ps = psum.tile([128, ACT_CHUNK], F32, tag="ps")
                for jj in range(ACT_CHUNK // BANK):
                    sl = slice(a * ACT_CHUNK + jj * BANK,
                               a * ACT_CHUNK + (jj + 1) * BANK)
                    nc.tensor.matmul(
                        ps[:, jj * BANK:(jj + 1) * BANK], lhs, k_sb[:, sl],
                        start=True, stop=True,
                    )
                # exp(SCALE * sim) with fused partial row-sum
                nc.scalar.activation(
                    out=out_grp[:, t, a * ACT_CHUNK:(a + 1) * ACT_CHUNK],
                    in_=ps,
                    func=mybir.ActivationFunctionType.Exp,
                    scale=SCALE,
                    accum_out=sums[:, a:a + 1],
                )
            rsum = small.tile([128, 1], F32, tag="rsum")
            nc.vector.tensor_reduce(
                out=rsum, in_=sums, axis=mybir.AxisListType.X,
                op=mybir.AluOpType.add,
            )
            recip = small.tile([128, 1], F32, tag="recip")
            nc.vector.reciprocal(out=recip, in_=rsum)
            nc.vector.tensor_scalar_mul(
                out=out_grp[:, t, :], in0=out_grp[:, t, :], scalar1=recip
            )
        nc.sync.dma_start(out=out_view[g], in_=out_grp)

    # x loaded with an SWDGE cast straight to fp32r, in halves: the
    # second half lives in the long-lived pool (16 KB/partition), the
    # first half in a scoped pool freed before output staging opens.
    x_view = x_ap.rearrange("(t p) s -> p t s", p=128)
    x1_sb = data.tile([128, 2, ACT_CHUNK], MM_DT)
    with tc.tile_pool(name="xpool", bufs=1) as xp:
        x0_sb = xp.tile([128, 2, ACT_CHUNK], MM_DT)
        for half, dst_x in ((0, x0_sb), (1, x1_sb)):
            for c in range(ACT_CHUNK // X_CHUNK):
                src = slice(half * ACT_CHUNK + c * X_CHUNK,
                            half * ACT_CHUNK + (c + 1) * X_CHUNK)
                loc = slice(c * X_CHUNK, (c + 1) * X_CHUNK)
                nc.gpsimd.dma_start(
                    out=dst_x[:, :, loc], in_=x_view[:, :, src]
                )

        # ---- projection, interleaved with the first attention groups.
        # Groups 0-7 read only q chunk 0, so q chunk 1 is deferred until
        # after them: the PE (in-order) reaches the first sim matmuls and
        # the first output DMA sooner.
        proj_chunk(1, k_sb, 0, x0_sb)   # k cols 0:2048
        proj_chunk(0, q_sb, 0, x0_sb)   # q rows 0:2048 (row-tiles 0-15)
        proj_chunk(1, k_sb, 1, x1_sb)   # k cols 2048:4096

    outp = ctx.enter_context(tc.tile_pool(name="outp", bufs=OUT_BUFS))
    for g in range(N_GRP // 2):
        emit_group(g)
        pace(PACE_ACTS)
    proj_chunk(0, q_sb, 1, x1_sb)       # q rows 2048:4096 (row-tiles 16-31)
    for g in range(N_GRP // 2, N_GRP):
        emit_group(g)
        if g < N_GRP - 1:
            pace(PACE_ACTS)


_built = None


def _get_nc():
    global _built
    if _built is None:
        nc = bacc.Bacc("TRN2", target_bir_lowering=False, debug=False)
        x = nc.dram_tensor("x", [C, HW], F32, kind="ExternalInput").ap()
        w = nc.dram_tensor("w", [2 * D, C], F32, kind="ExternalInput").ap()
        out = nc.dram_tensor("out", [HW, HW], F32, kind="ExternalOutput").ap()
        with tile.TileContext(nc) as tc:
            with ExitStack() as ctx:
                _emit(ctx, tc, out, x, w)
        nc.compile()
        _built = nc
    return _built


def kernel(x: np.ndarray, W: np.ndarray) -> np.ndarray:
    nc = _get_nc()
    x = np.asarray(x, dtype=np.float32)
    W = np.ascontiguousarray(np.asarray(W, dtype=np.float32))
    in_maps = [
        {"x": np.ascontiguousarray(x[b].reshape(C, HW)), "w": W} for b in range(B)
    ]
    res = run_bass_kernel_spmd(nc, in_maps, core_ids=list(range(N_CORES)))
    out = np.stack([res.results[b]["out"] for b in range(B)])
    return out[:, None]


# revision 26
# speedup vs baseline: 1.0381x; 1.0112x over previous
"""Spatial self-attention scores kernel for Trainium2 (8 NeuronCores).

Computes, per batch b:
    qk = W @ x_b          # [256, 4096] = [256,256] @ [256,4096]
    q, k = qk[:128], qk[128:]
    sim = (q.T @ k) * 128**-0.5
    out_b = softmax(sim, axis=-1)        # [4096, 4096]
Output: [8, 1, 4096, 4096] float32.

Sharding: data-parallel over batch, one batch image per NeuronCore.

Per-core pipeline (all phases overlap under the Tile scheduler):
  - x DMA'd in as float32r (SWDGE cast); W transposed on PE via identity.
  - projection matmuls -> q,k in SBUF as [d=128, s=4096] float32r.
  - per 128-query row-tile: 8 matmuls (K=128, N=512) into 4-bank PSUM
    tiles; one ScalarE ACTIVATE per 2048 columns computes exp(SCALE*sim)
    with a fused row-sum (accum_out); DVE combines the partial sums,
    takes the reciprocal, and scales the row in one tensor_scalar.
  - output rows DMA'd out in 4 MB blocks (two row-tiles per transfer).
"""

import numpy as np
from contextlib import ExitStack

import concourse.bass as bass
import concourse.tile as tile
from concourse import bacc, mybir
from concourse.bass_utils import run_bass_kernel_spmd
from concourse.masks import make_identity

B = 8
C = 256
HW = 4096
D = 128
SCALE = D ** -0.5
N_CORES = 8

BANK = 512             # PSUM bank width (fp32) = one matmul free-dim
ACT_CHUNK = 2048       # one ScalarE activation spans 4 banks
N_ACT = HW // ACT_CHUNK          # 2
GRP = 2                # row-tiles per output DMA (2 -> 4 MB transfers)
N_GRP = HW // (128 * GRP)        # 16
OUT_BUFS = 4
X_CHUNK = 1024         # x input DMA granularity (overlaps with projection)
# Dummy activations inserted between output groups (0 = disabled).
# Tried as HBM-demand pacing; run-to-run core imbalance turned out to be
# environmental, so pacing only taxed the fast cores.
PACE_ACTS = 0

F32 = mybir.dt.float32
# float32r streams through the PE at 2 cycles/row (vs 4 for float32)
# with near-fp32 precision (measured ~3e-4 scale-relative on this
# kernel). The BIR verifier requires fp32r operands to be *produced*
# as fp32r, so operand tiles carry this dtype and their producers
# (SWDGE cast DMA / DVE copies) write it directly.
MM_DT = mybir.dt.float32r


def _emit(ctx: ExitStack, tc: tile.TileContext, out_ap, x_ap, w_ap):
    nc = tc.nc

    const = ctx.enter_context(tc.tile_pool(name="const", bufs=1))
    data = ctx.enter_context(tc.tile_pool(name="data", bufs=1))
    psum = ctx.enter_context(tc.tile_pool(name="psum", bufs=2, space="PSUM"))
    small = ctx.enter_context(tc.tile_pool(name="small", bufs=4))

    ident = const.tile([128, 128], F32)
    make_identity(nc, ident)

    # ---- W [256, 256] -> SBUF as [p, o_tile, c]
    w_sb = const.tile([128, 2, C], F32)
    nc.sync.dma_start(out=w_sb, in_=w_ap.rearrange("(t p) c -> p t c", p=128))

    # ---- PE warm-up: ~7 us of throwaway matmuls while x is loading.
    # The PE clock gate (HAM) only releases to 2.4 GHz after ~3.4 us of
    # sustained activity; warming during the input DMA makes the
    # projection and the first attention row-tiles run at full rate.
    warm_f32 = const.tile([128, BANK], F32)
    nc.vector.memset(warm_f32, 0.0)
    warm = const.tile([128, BANK], MM_DT)
    nc.vector.tensor_copy(out=warm, in_=warm_f32)
    wps = psum.tile([128, ACT_CHUNK], F32, tag="ps")
    for _ in range(5):
        nc.tensor.matmul(
            wps[:, 0:BANK], warm[:, 0:128], warm, start=True, stop=True
        )
    # pull the exp table load off the first real activation
    tbl = small.tile([128, 1], F32, tag="tbl")
    nc.scalar.activation(
        out=tbl, in_=warm_f32[:, 0:1], func=mybir.ActivationFunctionType.Exp
    )
    pace_out = const.tile([128, BANK], F32)

    def pace(n):
        for _ in range(n):
            nc.scalar.activation(
                out=pace_out, in_=warm_f32,
                func=mybir.ActivationFunctionType.Copy,
            )

    # ---- transpose W on PE -> wt_sb[c_sub, c_tile, o] (contraction c on partitions)
    wt_sb = const.tile([128, 2, 2 * D], MM_DT)
    for t in range(2):          # output-channel tile (q half / k half)
        for ct in range(2):     # input-channel tile
            ps = psum.tile([128, ACT_CHUNK], F32, tag="ps")
            nc.tensor.transpose(
                ps[:, 0:128], w_sb[:, t, ct * 128:(ct + 1) * 128], ident
            )
            nc.vector.tensor_copy(
                out=wt_sb[:, ct, t * 128:(t + 1) * 128], in_=ps[:, 0:128]
            )

    q_sb = data.tile([128, HW], MM_DT)
    k_sb = data.tile([128, HW], MM_DT)

    def proj_chunk(t, dst, a, x_half):
        """Project output-channel half t for column chunk a; x_half holds
        x columns [a*ACT_CHUNK, (a+1)*ACT_CHUNK)."""
        ps = psum.tile([128, ACT_CHUNK], F32, tag="ps")
        for jj in range(ACT_CHUNK // BANK):
            sl = slice(a * ACT_CHUNK + jj * BANK,
                       a * ACT_CHUNK + (jj + 1) * BANK)
            lo = slice(jj * BANK, (jj + 1) * BANK)
            nc.tensor.matmul(
                ps[:, lo], wt_sb[:, 0, t * 128:(t + 1) * 128],
                x_half[:, 0, lo], start=True, stop=False,
            )
            nc.tensor.matmul(
                ps[:, lo], wt_sb[:, 1, t * 128:(t + 1) * 128],
                x_half[:, 1, lo], start=False, stop=True,
            )
            nc.vector.tensor_copy(out=dst[:, sl], in_=ps[:, lo])

    outp = None
    out_view = out_ap.rearrange("(g t p) m -> g p t m", t=GRP, p=128)

    def emit_group(g):
        out_grp = outp.tile([128, GRP, HW], F32, tag="out")
        for t in range(GRP):
            i = g * GRP + t
            lhs = q_sb[:, i * 128:(i + 1) * 128]
            sums = small.tile([128, N_ACT], F32, tag="sums")
            for a in range(N_ACT):
                ps = psum.tile([128, ACT_CHUNK], F32, tag="ps")
                for jj in range(ACT_CHUNK // BANK):
                    sl = slice(a * ACT_CHUNK + jj * BANK,
                               a * ACT_CHUNK + (jj + 1) * BANK)
                    nc.tensor.matmul(
                        ps[:, jj * BANK:(jj + 1) * BANK], lhs, k_sb[:, sl],
                        start=True, stop=True,
                    )
                # exp(SCALE * sim) with fused partial row-sum
                nc.scalar.activation(
                    out=out_grp[:, t, a * ACT_CHUNK:(a + 1) * ACT_CHUNK],
                    in_=ps,
                    func=mybir.ActivationFunctionType.Exp,
                    scale=SCALE,
                    accum_out=sums[:, a:a + 1],
                )
            rsum = small.tile([128, 1], F32, tag="rsum")
            nc.vector.tensor_reduce(
                out=rsum, in_=sums, axis=mybir.AxisListType.X,
                op=mybir.AluOpType.add,
            )
            recip = small.tile([128, 1], F32, tag="recip")
            nc.vector.reciprocal(out=recip, in_=rsum)
            nc.vector.tensor_scalar_mul(
                out=out_grp[:, t, :], in0=out_grp[:, t, :], scalar1=recip
            )
        nc.sync.dma_start(out=out_view[g], in_=out_grp)

    # x loaded with an SWDGE cast straight to fp32r, in halves: the
    # second half lives in the long-lived pool (16 KB/partition), the
    # first half in a scoped pool freed before output staging opens.
    x_view = x_ap.rearrange("(t p) s -> p t s", p=128)
    x1_sb = data.tile([128, 2, ACT_CHUNK], MM_DT)
    with tc.tile_pool(name="xpool", bufs=1) as xp:
        x0_sb = xp.tile([128, 2, ACT_CHUNK], MM_DT)
        for half, dst_x in ((0, x0_sb), (1, x1_sb)):
            for c in range(ACT_CHUNK // X_CHUNK):
                src = slice(half * ACT_CHUNK + c * X_CHUNK,
                            half * ACT_CHUNK + (c + 1) * X_CHUNK)
                loc = slice(c * X_CHUNK, (c + 1) * X_CHUNK)
                nc.gpsimd.dma_start(
                    out=dst_x[:, :, loc], in_=x_view[:, :, src]
                )

        # ---- projection, interleaved with the first attention groups.
        # Groups 0-7 read only q chunk 0, so q chunk 1 is deferred until
        # after them: the PE (in-order) reaches the first sim matmuls and
        # the first output DMA sooner.
        proj_chunk(1, k_sb, 0, x0_sb)   # k cols 0:2048
        proj_chunk(0, q_sb, 0, x0_sb)   # q rows 0:2048 (row-tiles 0-15)
        proj_chunk(1, k_sb, 1, x1_sb)   # k cols 2048:4096

    outp = ctx.enter_context(tc.tile_pool(name="outp", bufs=OUT_BUFS))
    for g in range(N_GRP // 2):
        emit_group(g)
        pace(PACE_ACTS)
    proj_chunk(0, q_sb, 1, x1_sb)       # q rows 2048:4096 (row-tiles 16-31)
    for g in range(N_GRP // 2, N_GRP):
        emit_group(g)
        if g < N_GRP - 1:
            pace(PACE_ACTS)


_built = None


def _get_nc():
    global _built
    if _built is None:
        nc = bacc.Bacc("TRN2", target_bir_lowering=False, debug=False)
        x = nc.dram_tensor("x", [C, HW], F32, kind="ExternalInput").ap()
        w = nc.dram_tensor("w", [2 * D, C], F32, kind="ExternalInput").ap()
        out = nc.dram_tensor("out", [HW, HW], F32, kind="ExternalOutput").ap()
        with tile.TileContext(nc) as tc:
            with ExitStack() as ctx:
                _emit(ctx, tc, out, x, w)
        nc.compile()
        _built = nc
    return _built


def kernel(x: np.ndarray, W: np.ndarray) -> np.ndarray:
    nc = _get_nc()
    x = np.asarray(x, dtype=np.float32)
    W = np.ascontiguousarray(np.asarray(W, dtype=np.float32))
    in_maps = [
        {"x": np.ascontiguousarray(x[b].reshape(C, HW)), "w": W} for b in range(B)
    ]
    res = run_bass_kernel_spmd(nc, in_maps, core_ids=list(range(N_CORES)))
    out = np.stack([res.results[b]["out"] for b in range(B)])
    return out[:, None]


# revision 30
# speedup vs baseline: 1.0704x; 1.0312x over previous
"""Spatial self-attention scores kernel for Trainium2 (8 NeuronCores).

Computes, per batch b:
    qk = W @ x_b          # [256, 4096] = [256,256] @ [256,4096]
    q, k = qk[:128], qk[128:]
    sim = (q.T @ k) * 128**-0.5
    out_b = softmax(sim, axis=-1)        # [4096, 4096]
Output: [8, 1, 4096, 4096] float32.

Sharding: data-parallel over batch, one batch image per NeuronCore.

Per-core pipeline (all phases overlap under the Tile scheduler):
  - x DMA'd in as float32r (SWDGE cast); W transposed on PE via identity.
  - projection matmuls -> q,k in SBUF as [d=128, s=4096] float32r.
  - per 128-query row-tile: 8 matmuls (K=128, N=512) into 4-bank PSUM
    tiles; one ScalarE ACTIVATE per 2048 columns computes exp(SCALE*sim)
    with a fused row-sum (accum_out); DVE combines the partial sums,
    takes the reciprocal, and scales the row in one tensor_scalar.
  - output rows DMA'd out in 4 MB blocks (two row-tiles per transfer).
"""

import numpy as np
from contextlib import ExitStack

import concourse.bass as bass
import concourse.tile as tile
from concourse import bacc, mybir
from concourse.bass_utils import run_bass_kernel_spmd
from concourse.masks import make_identity

B = 8
C = 256
HW = 4096
D = 128
SCALE = D ** -0.5
N_CORES = 8

BANK = 512             # PSUM bank width (fp32) = one matmul free-dim
ACT_CHUNK = 2048       # one ScalarE activation spans 4 banks
N_ACT = HW // ACT_CHUNK          # 2
GRP = 2                # row-tiles per output DMA (2 -> 4 MB transfers)
N_GRP = HW // (128 * GRP)        # 16
OUT_BUFS = 4
X_CHUNK = 1024         # x input DMA granularity (overlaps with projection)
# Dummy activations inserted between output groups (0 = disabled).
# Tried as HBM-demand pacing; run-to-run core imbalance turned out to be
# environmental, so pacing only taxed the fast cores.
PACE_ACTS = 0

F32 = mybir.dt.float32
# float32r streams through the PE at 2 cycles/row (vs 4 for float32)
# with near-fp32 precision (measured ~3e-4 scale-relative on this
# kernel). The BIR verifier requires fp32r operands to be *produced*
# as fp32r, so operand tiles carry this dtype and their producers
# (SWDGE cast DMA / DVE copies) write it directly.
MM_DT = mybir.dt.float32r


def _emit(ctx: ExitStack, tc: tile.TileContext, out_ap, x_ap, w_ap):
    nc = tc.nc

    const = ctx.enter_context(tc.tile_pool(name="const", bufs=1))
    data = ctx.enter_context(tc.tile_pool(name="data", bufs=1))
    psum = ctx.enter_context(tc.tile_pool(name="psum", bufs=2, space="PSUM"))
    small = ctx.enter_context(tc.tile_pool(name="small", bufs=4))

    # ---- PE warm-up: throwaway matmuls while x is loading. The PE
    # clock gate (HAM) only releases to 2.4 GHz after ~3.4 us of
    # sustained activity; warming during the input DMA makes the
    # projection and the first attention row-tiles run at full rate.
    warm_f32 = const.tile([128, BANK], F32)
    nc.vector.memset(warm_f32, 0.0)
    warm = const.tile([128, BANK], MM_DT)
    nc.vector.tensor_copy(out=warm, in_=warm_f32)
    wps = psum.tile([128, ACT_CHUNK], F32, tag="ps")
    for _ in range(5):
        nc.tensor.matmul(
            wps[:, 0:BANK], warm[:, 0:128], warm, start=True, stop=True
        )

    ident = const.tile([128, 128], F32)
    make_identity(nc, ident)

    # ---- W [256, 256] -> SBUF as [p, o_tile, c]
    w_sb = const.tile([128, 2, C], F32)
    nc.sync.dma_start(out=w_sb, in_=w_ap.rearrange("(t p) c -> p t c", p=128))
    # pull the exp table load off the first real activation
    tbl = small.tile([128, 1], F32, tag="tbl")
    nc.scalar.activation(
        out=tbl, in_=warm_f32[:, 0:1], func=mybir.ActivationFunctionType.Exp
    )
    pace_out = const.tile([128, BANK], F32)

    def pace(n):
        for _ in range(n):
            nc.scalar.activation(
                out=pace_out, in_=warm_f32,
                func=mybir.ActivationFunctionType.Copy,
            )

    # ---- transpose W on PE -> wt_sb[c_sub, c_tile, o] (contraction c on partitions)
    wt_sb = const.tile([128, 2, 2 * D], MM_DT)
    for t in range(2):          # output-channel tile (q half / k half)
        for ct in range(2):     # input-channel tile
            ps = psum.tile([128, ACT_CHUNK], F32, tag="ps")
            nc.tensor.transpose(
                ps[:, 0:128], w_sb[:, t, ct * 128:(ct + 1) * 128], ident
            )
            nc.vector.tensor_copy(
                out=wt_sb[:, ct, t * 128:(t + 1) * 128], in_=ps[:, 0:128]
            )

    q_sb = data.tile([128, HW], MM_DT)
    k_sb = data.tile([128, HW], MM_DT)

    def proj_chunk(t, dst, a, x_half):
        """Project output-channel half t for column chunk a; x_half holds
        x columns [a*ACT_CHUNK, (a+1)*ACT_CHUNK)."""
        ps = psum.tile([128, ACT_CHUNK], F32, tag="ps")
        for jj in range(ACT_CHUNK // BANK):
            sl = slice(a * ACT_CHUNK + jj * BANK,
                       a * ACT_CHUNK + (jj + 1) * BANK)
            lo = slice(jj * BANK, (jj + 1) * BANK)
            nc.tensor.matmul(
                ps[:, lo], wt_sb[:, 0, t * 128:(t + 1) * 128],
                x_half[:, 0, lo], start=True, stop=False,
            )
            nc.tensor.matmul(
                ps[:, lo], wt_sb[:, 1, t * 128:(t + 1) * 128],
                x_half[:, 1, lo], start=False, stop=True,
            )
            nc.vector.tensor_copy(out=dst[:, sl], in_=ps[:, lo])

    outp = None
    out_view = out_ap.rearrange("(g t p) m -> g p t m", t=GRP, p=128)

    def emit_group(g, split_dma=False):
        out_grp = outp.tile([128, GRP, HW], F32, tag="out")
        for t in range(GRP):
            i = g * GRP + t
            lhs = q_sb[:, i * 128:(i + 1) * 128]
            sums = small.tile([128, N_ACT], F32, tag="sums")
            for a in range(N_ACT):
                ps = psum.tile([128, ACT_CHUNK], F32, tag="ps")
                for jj in range(ACT_CHUNK // BANK):
                    sl = slice(a * ACT_CHUNK + jj * BANK,
                               a * ACT_CHUNK + (jj + 1) * BANK)
                    nc.tensor.matmul(
                        ps[:, jj * BANK:(jj + 1) * BANK], lhs, k_sb[:, sl],
                        start=True, stop=True,
                    )
                # exp(SCALE * sim) with fused partial row-sum
                nc.scalar.activation(
                    out=out_grp[:, t, a * ACT_CHUNK:(a + 1) * ACT_CHUNK],
                    in_=ps,
                    func=mybir.ActivationFunctionType.Exp,
                    scale=SCALE,
                    accum_out=sums[:, a:a + 1],
                )
            rsum = small.tile([128, 1], F32, tag="rsum")
            nc.vector.tensor_reduce(
                out=rsum, in_=sums, axis=mybir.AxisListType.X,
                op=mybir.AluOpType.add,
            )
            recip = small.tile([128, 1], F32, tag="recip")
            nc.vector.reciprocal(out=recip, in_=rsum)
            nc.vector.tensor_scalar_mul(
                out=out_grp[:, t, :], in0=out_grp[:, t, :], scalar1=recip
            )
            if split_dma:
                # ship each row-tile as its own 2 MB transfer so the
                # first/last outputs don't wait for their group partner
                i = g * GRP + t
                nc.sync.dma_start(
                    out=out_ap[i * 128:(i + 1) * 128, :], in_=out_grp[:, t, :]
                )
        if not split_dma:
            nc.sync.dma_start(out=out_view[g], in_=out_grp)

    # x loaded with an SWDGE cast straight to fp32r, in halves: the
    # second half lives in the long-lived pool (16 KB/partition), the
    # first half in a scoped pool freed before output staging opens.
    x_view = x_ap.rearrange("(t p) s -> p t s", p=128)
    x1_sb = data.tile([128, 2, ACT_CHUNK], MM_DT)
    with tc.tile_pool(name="xpool", bufs=1) as xp:
        x0_sb = xp.tile([128, 2, ACT_CHUNK], MM_DT)
        for half, dst_x in ((0, x0_sb), (1, x1_sb)):
            for c in range(ACT_CHUNK // X_CHUNK):
                src = slice(half * ACT_CHUNK + c * X_CHUNK,
                            half * ACT_CHUNK + (c + 1) * X_CHUNK)
                loc = slice(c * X_CHUNK, (c + 1) * X_CHUNK)
                nc.gpsimd.dma_start(
                    out=dst_x[:, :, loc], in_=x_view[:, :, src]
                )

        # ---- projection, interleaved with the first attention groups.
        # Groups 0-7 read only q chunk 0, so q chunk 1 is deferred until
        # after them: the PE (in-order) reaches the first sim matmuls and
        # the first output DMA sooner.
        proj_chunk(1, k_sb, 0, x0_sb)   # k cols 0:2048
        proj_chunk(0, q_sb, 0, x0_sb)   # q rows 0:2048 (row-tiles 0-15)
        proj_chunk(1, k_sb, 1, x1_sb)   # k cols 2048:4096

    outp = ctx.enter_context(tc.tile_pool(name="outp", bufs=OUT_BUFS))
    for g in range(N_GRP // 2):
        emit_group(g, split_dma=(g == 0))
        pace(PACE_ACTS)
    proj_chunk(0, q_sb, 1, x1_sb)       # q rows 2048:4096 (row-tiles 16-31)
    for g in range(N_GRP // 2, N_GRP):
        emit_group(g, split_dma=(g == N_GRP - 1))
        if g < N_GRP - 1:
            pace(PACE_ACTS)


_built = None


def _get_nc():
    global _built
    if _built is None:
        nc = bacc.Bacc("TRN2", target_bir_lowering=False, debug=False)
        x = nc.dram_tensor("x", [C, HW], F32, kind="ExternalInput").ap()
        w = nc.dram_tensor("w", [2 * D, C], F32, kind="ExternalInput").ap()
        out = nc.dram_tensor("out", [HW, HW], F32, kind="ExternalOutput").ap()
        with tile.TileContext(nc) as tc:
            with ExitStack() as ctx:
                _emit(ctx, tc, out, x, w)
        nc.compile()
        _built = nc
    return _built


def kernel(x: np.ndarray, W: np.ndarray) -> np.ndarray:
    nc = _get_nc()
    x = np.asarray(x, dtype=np.float32)
    W = np.ascontiguousarray(np.asarray(W, dtype=np.float32))
    in_maps = [
        {"x": np.ascontiguousarray(x[b].reshape(C, HW)), "w": W} for b in range(B)
    ]
    res = run_bass_kernel_spmd(nc, in_maps, core_ids=list(range(N_CORES)))
    out = np.stack([res.results[b]["out"] for b in range(B)])
    return out[:, None]
